# revision 37
# baseline (speedup 1.0000x reference)
"""Trainium2 Bass kernel for causal multi-head attention (B=4,S=2048,D=1024,N=16,H=64).

Sharding: 8 cores = (batch b in 0..3) x (head-group g in 0..1). Each core gets
residual[b] and 8 heads' worth of W_Q/K/V/O, computes the partial output
sum_{n in group} attn_n @ W_O[n]  ->  [2048,1024]; host adds the two
head-group partials per batch. No collectives needed.

Per-core layout strategy (all matmul operands fp16, fp32 PSUM accumulate):
  xt   [128, 8*2048]   X^T       (d on partitions)  - PE transposes
  wqt/wkt/wvt [128, 8*512] W^T   (d on partitions)  - PE transposes
  qt/kt [128, 4*2048]  Q^T/K^T   (2 heads per 128 partitions)
  v4   [128, 16*8*65]  V with a ones column per head (denominator trick)
  S^T tile = kt.T @ qt  ->  exp on ACT  ->  P^T (fp16, causal-masked)
  PV: out[sq,65] += P^T_tile.T @ V'_tile   (col 64 = softmax denominator)
  normalize on DVE (per-partition reciprocal broadcast), transpose AO on PE,
  O-projection fp16, DMA out fp32.
"""

import sys

sys.path.insert(0, "/opt/trn_rl_repo")

import numpy as np
import concourse.bass as bass
import concourse.mybir as mybir
import concourse.tile as tile
from concourse.bass_utils import run_bass_kernel_spmd
from concourse.masks import make_identity

F32 = mybir.dt.float32
F32R = mybir.dt.float32r
F16 = mybir.dt.float16
AF = mybir.ActivationFunctionType

S = 2048
D = 1024
NH = 8  # heads per core
H = 64
P = 128
ST = S // P  # 16
DT = D // P  # 8
NPAIR = NH // 2  # 4
SCALE = 1.0 / 8.0  # 1/sqrt(H)
INTERLEAVE = "chain"  # "chain" | "fine" | "none"
COMPUTE_MAX_WAITS = 1
PARTS = "full"  # "full" | "proj" | "noO" | "loads"
WLOAD = "hwdge"  # "hwdge" | "swdge"


CTRL_INSTS = ("InstDrain", "InstNop", "InstEventSemaphoreOp", "InstSemaphoreOp")


def split_excess_waits(nc, max_waits=1, compute_max_waits=1):
    """This walrus build rejects >1 sync wait on CTRL-class instructions
    (Drain/NoOp). Move excess waits onto same-engine NOPs inserted
    immediately before. Compute instructions may keep compute_max_waits."""
    n_split = 0
    for bb in nc.main_func.blocks:
        insts = list(bb.instructions)
        out = []
        for ins in insts:
            si = ins.sync_info
            lim = max_waits if type(ins).__name__ in CTRL_INSTS else compute_max_waits
            if si is not None and si.on_wait and len(si.on_wait) > lim:
                waits = list(si.on_wait)
                while len(waits) > lim:
                    chunk, waits = waits[:1], waits[1:]
                    nop = nc.engines[ins.engine].nop(nofuse=True).ins
                    for b2 in nc.main_func.blocks:
                        if nop in b2.instructions:
                            b2.instructions.remove(nop)
                            break
                    if nop.sync_info is None:
                        nop.sync_info = mybir.SyncInfo(on_wait=[], on_update=[])
                    nop.sync_info.on_wait = chunk
                    out.append(nop)
                    n_split += 1
                si.on_wait = waits
            out.append(ins)
        bb.instructions[:] = out
    return n_split


def emit(nc, tc, x, wq_d, wk_d, wv_d, wo_d, out_d, dbg=None):
    with (
        tc.tile_pool(name="const", bufs=1) as constp,
        tc.tile_pool(name="persist", bufs=1) as persist,
        tc.tile_pool(name="psM", bufs=2, space="PSUM") as psM,
    ):
        ident = constp.tile([P, P], F16)
        make_identity(nc, ident)
        identf = constp.tile([P, P], F32)
        make_identity(nc, identf)

        xt = persist.tile([P, DT * S], F16)
        wqt = persist.tile([P, DT * 512], F16)
        wkt = persist.tile([P, DT * 512], F16)
        wvt = persist.tile([P, DT * 512], F16)
        qt = persist.tile([P, NPAIR * S], F16)
        kt = persist.tile([P, NPAIR * S], F16)
        v4 = persist.tile([P, ST * NH * 65], F16)
        aot = persist.tile([P, NPAIR * S], F16)
        wo_sb = persist.tile([P, NPAIR * D], F16)

        v4v = v4.rearrange("p (i h e) -> p i h e", i=ST, h=NH)
        xtv = xt.rearrange("p (k s) -> p k s", k=DT)

        # projection-chain generators: yield after each matmul so the pair
        # loop can interleave single proj matmuls into ACT-bound gaps
        def gen_qk(wt_src, dst, c, t):
            pq = psM.tile([P, 512], F32, tag="big", bufs=3)
            for k in range(DT):
                nc.tensor.matmul(
                    pq,
                    lhsT=wt_src[:, k * 512 + t * P : k * 512 + (t + 1) * P],
                    rhs=xt[:, k * S + c * 512 : k * S + (c + 1) * 512],
                    start=(k == 0),
                    stop=(k == DT - 1),
                )
                yield
            nc.vector.tensor_copy(
                dst[:, t * S + c * 512 : t * S + (c + 1) * 512], pq
            )

        def gen_v(i, t):
            pv = psM.tile([P, 512], F32, tag="misc", bufs=3)
            for k in range(DT):
                nc.tensor.matmul(
                    pv[:, 0:P],
                    lhsT=xt[:, k * S + i * P : k * S + (i + 1) * P],
                    rhs=wvt[:, k * 512 + t * P : k * 512 + (t + 1) * P],
                    start=(k == 0),
                    stop=(k == DT - 1),
                )
                yield
            nc.vector.tensor_copy(
                v4v[:, i, 2 * t : 2 * t + 2, 0:64],
                pv[:, 0:P].rearrange("p (h e) -> p h e", h=2),
            )

        def em_qk(wt_src, dst, c, t):
            for _ in gen_qk(wt_src, dst, c, t):
                pass

        def em_v(i, t):
            for _ in gen_v(i, t):
                pass

        # ---- Phase 1: W^T (f16 SWDGE cast loads + f16 transposes) and X^T
        # (fp32 HWDGE loads + fp32r transposes, evacs cast to f16). X and W
        # transpose units interleave so PE is fed by whichever data arrived.
        with tc.tile_pool(name="stage", bufs=1) as stage:
            wfs_all = []
            if WLOAD == "hwdge":
                # one 2 MB dma_start per weight matrix: [512,1024] viewed as
                # [128, 4*1024] (partition-major) for fewer, larger descriptors
                for wd, wt in ((wq_d, wqt), (wk_d, wkt), (wv_d, wvt)):
                    wf = stage.tile([P, 4 * D], F32, tag="xf", bufs=3)
                    nc.sync.dma_start(
                        out=wf, in_=wd.rearrange("(a p) d -> p a d", p=P)
                    )
                    wfs_all.append((wf, wt))
                # W_O: HWDGE fp32 load + ACT cast to f16
                wof = stage.tile([P, 4 * D], F32, tag="xf", bufs=3)
                nc.sync.dma_start(
                    out=wof, in_=wo_d.rearrange("(a p) d -> p a d", p=P)
                )
                nc.scalar.copy(out=wo_sb, in_=wof)
            else:
                for wd, wt in ((wq_d, wqt), (wk_d, wkt), (wv_d, wvt)):
                    w16 = stage.tile([P, 4 * D], F16, tag="w16", bufs=3)
                    for j in range(4):
                        nc.gpsimd.dma_start(
                            out=w16[:, j * D : (j + 1) * D],
                            in_=wd[j * P : (j + 1) * P, :],
                        )
                    wfs_all.append((w16, wt))
                for j in range(NPAIR):
                    nc.gpsimd.dma_start(
                        out=wo_sb[:, j * D : (j + 1) * D],
                        in_=wo_d[j * P : (j + 1) * P, :],
                    )
            nc.gpsimd.memset(v4v[:, :, :, 64], 1.0)

            xf_by_g = {}

            def load_group(ig):
                # one 2 MB dma_start per 4-sq-tile group
                xf = stage.tile([P, 4 * D], F32, tag="xf", bufs=3)
                nc.sync.dma_start(
                    out=xf,
                    in_=x[ig * 4 * P : (ig + 1) * 4 * P, :].rearrange(
                        "(a p) d -> p a d", p=P
                    ),
                )
                xf_by_g[ig] = xf

            def x_unit(ig):
                xf = xf_by_g.pop(ig)
                for ii in range(4):
                    for kg in range(2):
                        pt = psM.tile([P, 512], F32, tag="big", bufs=3)
                        for kk in range(4):
                            k = 4 * kg + kk
                            nc.tensor.transpose(
                                pt[:, kk * P : (kk + 1) * P],
                                xf[:, ii * D + k * P : ii * D + (k + 1) * P],
                                identf,
                            )
                        nc.vector.tensor_copy(
                            xtv[:, 4 * kg : 4 * kg + 4, (4 * ig + ii) * P : (4 * ig + ii + 1) * P],
                            pt.rearrange("p (k c) -> p k c", k=4),
                        )

            def w_unit(widx, j):
                src_w, wt = wfs_all[widx]
                wtv = wt.rearrange("p (k c) -> p k c", k=DT)
                for kg in range(2):
                    if WLOAD == "hwdge":
                        wp = psM.tile([P, 512], F32, tag="big", bufs=3)
                    else:
                        wp = psM.tile([P, 512], F16, tag="big", bufs=3)
                    for kk in range(4):
                        k = 4 * kg + kk
                        if WLOAD == "hwdge":
                            nc.tensor.transpose(
                                wp[:, kk * P : (kk + 1) * P],
                                src_w[:, j * D + k * P : j * D + (k + 1) * P],
                                identf,
                            )
                        else:
                            nc.tensor.transpose(
                                wp[:, kk * P : (kk + 1) * P],
                                src_w[:, j * D + k * P : j * D + (k + 1) * P],
                                ident,
                            )
                    nc.scalar.copy(
                        out=wtv[:, 4 * kg : 4 * kg + 4, j * P : (j + 1) * P],
                        in_=wp.rearrange("p (k c) -> p k c", k=4),
                    )

            for ig in range(4):
                load_group(ig)
            if PARTS != "loads":
                for w in range(3):
                    for j in range(4):
                        w_unit(w, j)
                for ig in range(4):
                    x_unit(ig)
                # pair-0 projections
                for c in range(4):
                    em_qk(wqt, qt, c, 0)
                    em_qk(wkt, kt, c, 0)
                for i in range(ST):
                    em_v(i, 0)

        if dbg is not None:
            nc.gpsimd.dma_start(out=dbg["xt"][:], in_=xt)
            nc.gpsimd.dma_start(out=dbg["wqt"][:], in_=wqt)

        # ---- Main loop: per head pair, attention for its two heads.
        # Pair t+1's projections (and for the last pair, the O-projection)
        # interleave into the ACT-bound attention stream to keep PE busy.
        for t in range(NPAIR if PARTS != "loads" else 0):
            gens = []
            if t + 1 < NPAIR:
                for wt_src, dst in ((wqt, qt), (wkt, kt)):
                    for c in range(4):
                        gens.append(gen_qk(wt_src, dst, c, t + 1))
                for i in range(ST):
                    gens.append(gen_v(i, t + 1))
            gens.reverse()

            def pull(n):
                for _ in range(n):
                    while gens:
                        try:
                            next(gens[-1])
                            break
                        except StopIteration:
                            gens.pop()
                    if not gens:
                        return

            if PARTS == "proj":
                pull(10 ** 6)
                continue
            for G in (3, 2, 1, 0):
                njs = 4 * G + 4  # sk-tiles 0..4G+3
                tpq = psM.tile([P, 512], F16, tag="misc", bufs=3)
                for half in range(2):
                    if INTERLEAVE == "chain":
                        pull(24)  # three full proj chains per block
                    h = 2 * t + half
                    pb = 64 * half
                    po = psM.tile([P, 4 * 65], F32, tag="po")
                    sts = [None] * njs

                    def emit_st(j):
                        o = max(0, (j - 4 * G) * P)
                        st = psM.tile([P, 512], F32, tag="big", bufs=3)
                        nc.tensor.matmul(
                            st[:, o:512],
                            lhsT=kt[
                                pb : pb + 64, t * S + j * P : t * S + (j + 1) * P
                            ],
                            rhs=qt[
                                pb : pb + 64,
                                t * S + G * 512 + o : t * S + (G + 1) * 512,
                            ],
                            start=True,
                            stop=True,
                        )
                        sts[j] = (st, o)

                    emit_st(0)
                    for j in range(njs):
                        if j + 1 < njs:
                            emit_st(j + 1)  # pipeline: next S^T before this PV
                        if INTERLEAVE == "fine":
                            pull(3)
                        st, o = sts[j]
                        sts[j] = None
                        pts = work_tile(persist, "pts", [P, 512], F16, 6)
                        nc.scalar.activation(
                            pts[:, o:512], st[:, o:512], AF.Exp, scale=SCALE
                        )
                        if j >= 4 * G:
                            # diagonal tile: keep q >= k (col >= partition)
                            nc.gpsimd.affine_select(
                                out=pts[:, o : o + P],
                                in_=pts[:, o : o + P],
                                compare_op=mybir.AluOpType.is_ge,
                                fill=0.0,
                                base=0,
                                pattern=[[1, P]],
                                channel_multiplier=-1,
                            )
                        for ii in range(4):
                            i = 4 * G + ii
                            if i < j:
                                continue
                            # PSUM start=True clears has_written for the WHOLE
                            # bank, so only the first matmul of the tile sets
                            # it; later regions first-write onto cleared
                            # has_written (= overwrite).
                            nc.tensor.matmul(
                                po[:, ii * 65 : (ii + 1) * 65],
                                lhsT=pts[:, ii * P : (ii + 1) * P],
                                rhs=v4v[:, j, h, :],
                                start=(j == 0 and ii == 0),
                                stop=(j == i),
                                skip_group_check=True,
                            )
                    # normalize: aon = po[:, i, 0:64] * (1/po[:, i, 64])
                    pov = po.rearrange("p (i e) -> p i e", i=4)
                    rec = work_tile(persist, "rec", [P, 4], F32, 2)
                    nc.vector.reciprocal(rec, pov[:, :, 64])
                    aon = work_tile(persist, "aon", [P, 256], F16, 2)
                    nc.vector.tensor_tensor(
                        out=aon.rearrange("p (i e) -> p i e", i=4),
                        in0=pov[:, :, 0:64],
                        in1=rec.to_broadcast((P, 4, 64)),
                        op=mybir.AluOpType.mult,
                    )
                    # transpose AO [sq,64] -> [64,sq] into partition half pb
                    for ii in range(4):
                        nc.tensor.transpose(
                            tpq[pb : pb + 64, ii * P : (ii + 1) * P],
                            aon[:, ii * 64 : (ii + 1) * 64],
                            ident,
                        )
                nc.vector.tensor_copy(
                    aot[:, t * S + G * 512 : t * S + (G + 1) * 512], tpq
                )
                if t == NPAIR - 1 and PARTS != "noO":
                    # O-projection for this sq group, interleaved into the
                    # last pair's ACT-bound attention stream; both 512-col
                    # chunks batched into one 512KB store per row tile
                    for ii in range(4):
                        i = 4 * G + ii
                        osb = work_tile(persist, "osb", [P, D], F32, 3)
                        for c in range(2):
                            oo = psM.tile([P, 512], F32, tag="misc", bufs=3)
                            for tp in range(NPAIR):
                                nc.tensor.matmul(
                                    oo,
                                    lhsT=aot[
                                        :, tp * S + i * P : tp * S + (i + 1) * P
                                    ],
                                    rhs=wo_sb[
                                        :, tp * D + c * 512 : tp * D + (c + 1) * 512
                                    ],
                                    start=(tp == 0),
                                    stop=(tp == NPAIR - 1),
                                )
                            nc.vector.tensor_copy(
                                osb[:, c * 512 : (c + 1) * 512], oo
                            )
                        nc.sync.dma_start(
                            out=out_d[i * P : (i + 1) * P, :], in_=osb
                        )
            pull(10 ** 6)  # drain any remaining proj work for pair t+1

        if dbg is not None:
            nc.gpsimd.dma_start(out=dbg["qt"][:], in_=qt)
            nc.gpsimd.dma_start(out=dbg["kt"][:], in_=kt)
            nc.gpsimd.dma_start(out=dbg["v4"][:], in_=v4)
            nc.gpsimd.dma_start(out=dbg["aot"][:], in_=aot)


def work_tile(pool, tag, shape, dtype, bufs):
    wt = pool.tile(shape, dtype, tag=tag, bufs=bufs, name=tag)
    return wt


def build_nc(debug=False):
    nc = bass.Bass()
    x = nc.dram_tensor("x", [S, D], F32, kind="ExternalInput")
    wq_d = nc.dram_tensor("wq", [NH * H, D], F32, kind="ExternalInput")
    wk_d = nc.dram_tensor("wk", [NH * H, D], F32, kind="ExternalInput")
    wv_d = nc.dram_tensor("wv", [NH * H, D], F32, kind="ExternalInput")
    wo_d = nc.dram_tensor("wo", [NH * H, D], F32, kind="ExternalInput")
    out_d = nc.dram_tensor("out", [S, D], F32, kind="ExternalOutput")
    dbg = None
    if debug:
        dbg = {
            "xt": nc.dram_tensor("dbg_xt", [P, DT * S], F32, kind="ExternalOutput"),
            "wqt": nc.dram_tensor("dbg_wqt", [P, DT * 512], F32, kind="ExternalOutput"),
            "qt": nc.dram_tensor("dbg_qt", [P, NPAIR * S], F32, kind="ExternalOutput"),
            "kt": nc.dram_tensor("dbg_kt", [P, NPAIR * S], F32, kind="ExternalOutput"),
            "v4": nc.dram_tensor("dbg_v4", [P, ST * NH * 65], F32, kind="ExternalOutput"),
            "aot": nc.dram_tensor("dbg_aot", [P, NPAIR * S], F32, kind="ExternalOutput"),
        }
    with tile.TileContext(nc) as tc:
        emit(nc, tc, x, wq_d, wk_d, wv_d, wo_d, out_d, dbg=dbg)
    split_excess_waits(nc, compute_max_waits=COMPUTE_MAX_WAITS)
    return nc


_cache = {}


def _get_runner():
    """Persistent jitted 8-core runner (mirrors bass2jax.run_bass_via_pjrt's
    multi-core path, but reusable across calls so we can time executions)."""
    if "runner" in _cache:
        return _cache["runner"]
    import jax
    from jax.experimental.shard_map import shard_map
    from jax.sharding import Mesh, PartitionSpec
    from concourse import bass2jax

    bass2jax.install_neuronx_cc_hook()
    if "nc" not in _cache:
        _cache["nc"] = build_nc()
    nc = _cache["nc"]

    partition_name = nc.partition_id_tensor.name if nc.partition_id_tensor else None
    in_names, out_names, out_avals = [], [], []
    for alloc in nc.m.functions[0].allocations:
        if not isinstance(alloc, mybir.MemoryLocationSet):
            continue
        name = alloc.memorylocations[0].name
        if alloc.kind == "ExternalInput":
            if name != partition_name:
                in_names.append(name)
        elif alloc.kind == "ExternalOutput":
            out_names.append(name)
            out_avals.append(
                jax.core.ShapedArray(tuple(alloc.tensor_shape), mybir.dt.np(alloc.dtype))
            )
    n_params, n_outs = len(in_names), len(out_names)
    all_names = list(in_names) + list(out_names)
    if partition_name is not None:
        all_names.append(partition_name)
    all_names = tuple(all_names)

    def _body(*args):
        operands = list(args)
        if partition_name is not None:
            operands.append(bass2jax.partition_id_tensor())
        outs = bass2jax._bass_exec_p.bind(
            *operands,
            out_avals=tuple(out_avals),
            in_names=all_names,
            out_names=tuple(out_names),
            lowering_input_output_aliases=(),
            sim_require_finite=True,
            sim_require_nnan=True,
            nc=nc,
        )
        return tuple(outs)

    devices = jax.devices()[:8]
    mesh = Mesh(np.asarray(devices), ("core",))
    in_specs = (PartitionSpec("core"),) * (n_params + n_outs)
    out_specs = (PartitionSpec("core"),) * n_outs
    donate = tuple(range(n_params, n_params + n_outs))
    sharded = jax.jit(
        shard_map(_body, mesh=mesh, in_specs=in_specs, out_specs=out_specs, check_rep=False),
        donate_argnums=donate,
        keep_unused=True,
    )
    _cache["runner"] = (sharded, in_names, out_names, out_avals, mesh)
    return _cache["runner"]


def run_on_cores(in_maps):
    """Run the kernel on 8 cores; returns list of per-core output dicts."""
    sharded, in_names, out_names, out_avals, mesh = _get_runner()
    concat_in = [
        np.concatenate([np.asarray(in_maps[c][name]) for c in range(8)], axis=0)
        for name in in_names
    ]
    concat_zeros = [
        np.zeros((8 * a.shape[0], *a.shape[1:]), a.dtype) for a in out_avals
    ]
    out_arrs = sharded(*concat_in, *concat_zeros)
    return [
        {
            name: np.asarray(out_arrs[i]).reshape(8, *out_avals[i].shape)[c]
            for i, name in enumerate(out_names)
        }
        for c in range(8)
    ]


def make_in_maps(residual, W_Q, W_K, W_V, W_O):
    in_maps = []
    for core in range(8):
        b, g = core // 2, core % 2
        sl = slice(8 * g, 8 * (g + 1))
        in_maps.append(
            {
                "x": np.ascontiguousarray(residual[b], dtype=np.float32),
                "wq": np.ascontiguousarray(W_Q[sl].reshape(NH * H, D), dtype=np.float32),
                "wk": np.ascontiguousarray(W_K[sl].reshape(NH * H, D), dtype=np.float32),
                "wv": np.ascontiguousarray(W_V[sl].reshape(NH * H, D), dtype=np.float32),
                "wo": np.ascontiguousarray(W_O[sl].reshape(NH * H, D), dtype=np.float32),
            }
        )
    return in_maps


def kernel(residual, W_Q, W_K, W_V, W_O):
    residual = np.asarray(residual)
    in_maps = make_in_maps(residual, W_Q, W_K, W_V, W_O)
    results = run_on_cores(in_maps)
    B = residual.shape[0]
    out = np.zeros((B, S, D), np.float32)
    for core in range(8):
        b = core // 2
        out[b] += results[core]["out"]
    return out


if __name__ == "__main__":
    rng = np.random.default_rng(0)
    residual = rng.standard_normal((4, S, D)).astype(np.float32)
    W = [0.02 * rng.standard_normal((16, H, D)).astype(np.float32) for _ in range(4)]
    out = kernel(residual, *W)
    print("kernel ran, out shape", out.shape, "finite:", np.isfinite(out).all())


# revision 38
# speedup vs baseline: 1.0296x; 1.0296x over previous
"""Trainium2 Bass kernel for causal multi-head attention (B=4,S=2048,D=1024,N=16,H=64).

Sharding: 8 cores = (batch b in 0..3) x (head-group g in 0..1). Each core gets
residual[b] and 8 heads' worth of W_Q/K/V/O, computes the partial output
sum_{n in group} attn_n @ W_O[n]  ->  [2048,1024]; host adds the two
head-group partials per batch. No collectives needed.

Per-core layout strategy (all matmul operands fp16, fp32 PSUM accumulate):
  xt   [128, 8*2048]   X^T       (d on partitions)  - PE transposes
  wqt/wkt/wvt [128, 8*512] W^T   (d on partitions)  - PE transposes
  qt/kt [128, 4*2048]  Q^T/K^T   (2 heads per 128 partitions)
  v4   [128, 16*8*65]  V with a ones column per head (denominator trick)
  S^T tile = kt.T @ qt  ->  exp on ACT  ->  P^T (fp16, causal-masked)
  PV: out[sq,65] += P^T_tile.T @ V'_tile   (col 64 = softmax denominator)
  normalize on DVE (per-partition reciprocal broadcast), transpose AO on PE,
  O-projection fp16, DMA out fp32.
"""

import sys

sys.path.insert(0, "/opt/trn_rl_repo")

import numpy as np
import concourse.bass as bass
import concourse.mybir as mybir
import concourse.tile as tile
from concourse.bass_utils import run_bass_kernel_spmd
from concourse.masks import make_identity

F32 = mybir.dt.float32
F32R = mybir.dt.float32r
F16 = mybir.dt.float16
AF = mybir.ActivationFunctionType

S = 2048
D = 1024
NH = 8  # heads per core
H = 64
P = 128
ST = S // P  # 16
DT = D // P  # 8
NPAIR = NH // 2  # 4
SCALE = 1.0 / 8.0  # 1/sqrt(H)
INTERLEAVE = "chain"  # "chain" | "fine" | "none"
COMPUTE_MAX_WAITS = 1
PARTS = "full"  # "full" | "proj" | "noO" | "loads"
WLOAD = "hwdge"  # "hwdge" | "swdge"


CTRL_INSTS = ("InstDrain", "InstNop", "InstEventSemaphoreOp", "InstSemaphoreOp")


def split_excess_waits(nc, max_waits=1, compute_max_waits=1):
    """This walrus build rejects >1 sync wait on CTRL-class instructions
    (Drain/NoOp). Move excess waits onto same-engine NOPs inserted
    immediately before. Compute instructions may keep compute_max_waits."""
    n_split = 0
    for bb in nc.main_func.blocks:
        insts = list(bb.instructions)
        out = []
        for ins in insts:
            si = ins.sync_info
            lim = max_waits if type(ins).__name__ in CTRL_INSTS else compute_max_waits
            if si is not None and si.on_wait and len(si.on_wait) > lim:
                waits = list(si.on_wait)
                while len(waits) > lim:
                    chunk, waits = waits[:1], waits[1:]
                    nop = nc.engines[ins.engine].nop(nofuse=True).ins
                    for b2 in nc.main_func.blocks:
                        if nop in b2.instructions:
                            b2.instructions.remove(nop)
                            break
                    if nop.sync_info is None:
                        nop.sync_info = mybir.SyncInfo(on_wait=[], on_update=[])
                    nop.sync_info.on_wait = chunk
                    out.append(nop)
                    n_split += 1
                si.on_wait = waits
            out.append(ins)
        bb.instructions[:] = out
    return n_split


def emit(nc, tc, x, wq_d, wk_d, wv_d, wo_d, out_d, dbg=None):
    with (
        tc.tile_pool(name="const", bufs=1) as constp,
        tc.tile_pool(name="persist", bufs=1) as persist,
        tc.tile_pool(name="psM", bufs=2, space="PSUM") as psM,
    ):
        ident = constp.tile([P, P], F16)
        make_identity(nc, ident)
        identf = constp.tile([P, P], F32)
        make_identity(nc, identf)

        xt = persist.tile([P, DT * S], F16)
        wqt = persist.tile([P, DT * 512], F16)
        wkt = persist.tile([P, DT * 512], F16)
        wvt = persist.tile([P, DT * 512], F16)
        qt = persist.tile([P, NPAIR * S], F16)
        kt = persist.tile([P, NPAIR * S], F16)
        v4 = persist.tile([P, ST * NH * 65], F16)
        aot = persist.tile([P, NPAIR * S], F16)
        wo_sb = persist.tile([P, NPAIR * D], F16)

        v4v = v4.rearrange("p (i h e) -> p i h e", i=ST, h=NH)
        xtv = xt.rearrange("p (k s) -> p k s", k=DT)

        # projection-chain generators: yield after each matmul so the pair
        # loop can interleave single proj matmuls into ACT-bound gaps
        def gen_qk(wt_src, dst, c, t):
            pq = psM.tile([P, 512], F32, tag="big", bufs=3)
            for k in range(DT):
                nc.tensor.matmul(
                    pq,
                    lhsT=wt_src[:, k * 512 + t * P : k * 512 + (t + 1) * P],
                    rhs=xt[:, k * S + c * 512 : k * S + (c + 1) * 512],
                    start=(k == 0),
                    stop=(k == DT - 1),
                )
                yield
            nc.vector.tensor_copy(
                dst[:, t * S + c * 512 : t * S + (c + 1) * 512], pq
            )

        def gen_v(i, t):
            pv = psM.tile([P, 512], F32, tag="misc", bufs=3)
            for k in range(DT):
                nc.tensor.matmul(
                    pv[:, 0:P],
                    lhsT=xt[:, k * S + i * P : k * S + (i + 1) * P],
                    rhs=wvt[:, k * 512 + t * P : k * 512 + (t + 1) * P],
                    start=(k == 0),
                    stop=(k == DT - 1),
                )
                yield
            nc.vector.tensor_copy(
                v4v[:, i, 2 * t : 2 * t + 2, 0:64],
                pv[:, 0:P].rearrange("p (h e) -> p h e", h=2),
            )

        def em_qk(wt_src, dst, c, t):
            for _ in gen_qk(wt_src, dst, c, t):
                pass

        def em_v(i, t):
            for _ in gen_v(i, t):
                pass

        # ---- Phase 1: W^T (f16 SWDGE cast loads + f16 transposes) and X^T
        # (fp32 HWDGE loads + fp32r transposes, evacs cast to f16). X and W
        # transpose units interleave so PE is fed by whichever data arrived.
        with tc.tile_pool(name="stage", bufs=1) as stage:
            wfs_all = []
            if WLOAD == "hwdge":
                # one 2 MB dma_start per weight matrix: [512,1024] viewed as
                # [128, 4*1024] (partition-major) for fewer, larger descriptors
                for wi, (wd, wt) in enumerate(((wq_d, wqt), (wk_d, wkt), (wv_d, wvt))):
                    wf = stage.tile([P, 4 * D], F32, tag="xf", bufs=3)
                    eng = nc.sync if wi % 2 == 0 else nc.scalar
                    eng.dma_start(
                        out=wf, in_=wd.rearrange("(a p) d -> p a d", p=P)
                    )
                    wfs_all.append((wf, wt))
                # W_O: HWDGE fp32 load + ACT cast to f16
                wof = stage.tile([P, 4 * D], F32, tag="xf", bufs=3)
                nc.scalar.dma_start(
                    out=wof, in_=wo_d.rearrange("(a p) d -> p a d", p=P)
                )
                nc.scalar.copy(out=wo_sb, in_=wof)
            else:
                for wd, wt in ((wq_d, wqt), (wk_d, wkt), (wv_d, wvt)):
                    w16 = stage.tile([P, 4 * D], F16, tag="w16", bufs=3)
                    for j in range(4):
                        nc.gpsimd.dma_start(
                            out=w16[:, j * D : (j + 1) * D],
                            in_=wd[j * P : (j + 1) * P, :],
                        )
                    wfs_all.append((w16, wt))
                for j in range(NPAIR):
                    nc.gpsimd.dma_start(
                        out=wo_sb[:, j * D : (j + 1) * D],
                        in_=wo_d[j * P : (j + 1) * P, :],
                    )
            nc.gpsimd.memset(v4v[:, :, :, 64], 1.0)

            xf_by_g = {}

            def load_group(ig):
                # one 2 MB dma_start per 4-sq-tile group
                xf = stage.tile([P, 4 * D], F32, tag="xf", bufs=3)
                eng = nc.sync if ig % 2 == 0 else nc.scalar
                eng.dma_start(
                    out=xf,
                    in_=x[ig * 4 * P : (ig + 1) * 4 * P, :].rearrange(
                        "(a p) d -> p a d", p=P
                    ),
                )
                xf_by_g[ig] = xf

            def x_unit(ig):
                xf = xf_by_g.pop(ig)
                for ii in range(4):
                    for kg in range(2):
                        pt = psM.tile([P, 512], F32, tag="big", bufs=3)
                        for kk in range(4):
                            k = 4 * kg + kk
                            nc.tensor.transpose(
                                pt[:, kk * P : (kk + 1) * P],
                                xf[:, ii * D + k * P : ii * D + (k + 1) * P],
                                identf,
                            )
                        nc.vector.tensor_copy(
                            xtv[:, 4 * kg : 4 * kg + 4, (4 * ig + ii) * P : (4 * ig + ii + 1) * P],
                            pt.rearrange("p (k c) -> p k c", k=4),
                        )

            def w_unit(widx, j):
                src_w, wt = wfs_all[widx]
                wtv = wt.rearrange("p (k c) -> p k c", k=DT)
                for kg in range(2):
                    if WLOAD == "hwdge":
                        wp = psM.tile([P, 512], F32, tag="big", bufs=3)
                    else:
                        wp = psM.tile([P, 512], F16, tag="big", bufs=3)
                    for kk in range(4):
                        k = 4 * kg + kk
                        if WLOAD == "hwdge":
                            nc.tensor.transpose(
                                wp[:, kk * P : (kk + 1) * P],
                                src_w[:, j * D + k * P : j * D + (k + 1) * P],
                                identf,
                            )
                        else:
                            nc.tensor.transpose(
                                wp[:, kk * P : (kk + 1) * P],
                                src_w[:, j * D + k * P : j * D + (k + 1) * P],
                                ident,
                            )
                    nc.scalar.copy(
                        out=wtv[:, 4 * kg : 4 * kg + 4, j * P : (j + 1) * P],
                        in_=wp.rearrange("p (k c) -> p k c", k=4),
                    )

            for ig in range(4):
                load_group(ig)
            if PARTS != "loads":
                for w in range(3):
                    for j in range(4):
                        w_unit(w, j)
                for ig in range(4):
                    x_unit(ig)
                # pair-0 projections
                for c in range(4):
                    em_qk(wqt, qt, c, 0)
                    em_qk(wkt, kt, c, 0)
                for i in range(ST):
                    em_v(i, 0)

        if dbg is not None:
            nc.gpsimd.dma_start(out=dbg["xt"][:], in_=xt)
            nc.gpsimd.dma_start(out=dbg["wqt"][:], in_=wqt)

        # ---- Main loop: per head pair, attention for its two heads.
        # Pair t+1's projections (and for the last pair, the O-projection)
        # interleave into the ACT-bound attention stream to keep PE busy.
        for t in range(NPAIR if PARTS != "loads" else 0):
            gens = []
            if t + 1 < NPAIR:
                for wt_src, dst in ((wqt, qt), (wkt, kt)):
                    for c in range(4):
                        gens.append(gen_qk(wt_src, dst, c, t + 1))
                for i in range(ST):
                    gens.append(gen_v(i, t + 1))
            gens.reverse()

            def pull(n):
                for _ in range(n):
                    while gens:
                        try:
                            next(gens[-1])
                            break
                        except StopIteration:
                            gens.pop()
                    if not gens:
                        return

            if PARTS == "proj":
                pull(10 ** 6)
                continue
            for G in (3, 2, 1, 0):
                njs = 4 * G + 4  # sk-tiles 0..4G+3
                tpq = psM.tile([P, 512], F16, tag="misc", bufs=3)
                for half in range(2):
                    if INTERLEAVE == "chain":
                        pull(24)  # three full proj chains per block
                    h = 2 * t + half
                    pb = 64 * half
                    po = psM.tile([P, 4 * 65], F32, tag="po")
                    sts = [None] * njs

                    def emit_st(j):
                        o = max(0, (j - 4 * G) * P)
                        st = psM.tile([P, 512], F32, tag="big", bufs=3)
                        nc.tensor.matmul(
                            st[:, o:512],
                            lhsT=kt[
                                pb : pb + 64, t * S + j * P : t * S + (j + 1) * P
                            ],
                            rhs=qt[
                                pb : pb + 64,
                                t * S + G * 512 + o : t * S + (G + 1) * 512,
                            ],
                            start=True,
                            stop=True,
                        )
                        sts[j] = (st, o)

                    emit_st(0)
                    for j in range(njs):
                        if j + 1 < njs:
                            emit_st(j + 1)  # pipeline: next S^T before this PV
                        if INTERLEAVE == "fine":
                            pull(3)
                        st, o = sts[j]
                        sts[j] = None
                        pts = work_tile(persist, "pts", [P, 512], F16, 6)
                        nc.scalar.activation(
                            pts[:, o:512], st[:, o:512], AF.Exp, scale=SCALE
                        )
                        if j >= 4 * G:
                            # diagonal tile: keep q >= k (col >= partition)
                            nc.gpsimd.affine_select(
                                out=pts[:, o : o + P],
                                in_=pts[:, o : o + P],
                                compare_op=mybir.AluOpType.is_ge,
                                fill=0.0,
                                base=0,
                                pattern=[[1, P]],
                                channel_multiplier=-1,
                            )
                        for ii in range(4):
                            i = 4 * G + ii
                            if i < j:
                                continue
                            # PSUM start=True clears has_written for the WHOLE
                            # bank, so only the first matmul of the tile sets
                            # it; later regions first-write onto cleared
                            # has_written (= overwrite).
                            nc.tensor.matmul(
                                po[:, ii * 65 : (ii + 1) * 65],
                                lhsT=pts[:, ii * P : (ii + 1) * P],
                                rhs=v4v[:, j, h, :],
                                start=(j == 0 and ii == 0),
                                stop=(j == i),
                                skip_group_check=True,
                            )
                    # normalize: aon = po[:, i, 0:64] * (1/po[:, i, 64])
                    pov = po.rearrange("p (i e) -> p i e", i=4)
                    rec = work_tile(persist, "rec", [P, 4], F32, 2)
                    nc.vector.reciprocal(rec, pov[:, :, 64])
                    aon = work_tile(persist, "aon", [P, 256], F16, 2)
                    nc.vector.tensor_tensor(
                        out=aon.rearrange("p (i e) -> p i e", i=4),
                        in0=pov[:, :, 0:64],
                        in1=rec.to_broadcast((P, 4, 64)),
                        op=mybir.AluOpType.mult,
                    )
                    # transpose AO [sq,64] -> [64,sq] into partition half pb
                    for ii in range(4):
                        nc.tensor.transpose(
                            tpq[pb : pb + 64, ii * P : (ii + 1) * P],
                            aon[:, ii * 64 : (ii + 1) * 64],
                            ident,
                        )
                nc.vector.tensor_copy(
                    aot[:, t * S + G * 512 : t * S + (G + 1) * 512], tpq
                )
                if t == NPAIR - 1 and PARTS != "noO":
                    # O-projection for this sq group, interleaved into the
                    # last pair's ACT-bound attention stream; both 512-col
                    # chunks batched into one 512KB store per row tile
                    for ii in range(4):
                        i = 4 * G + ii
                        osb = work_tile(persist, "osb", [P, D], F32, 3)
                        for c in range(2):
                            oo = psM.tile([P, 512], F32, tag="misc", bufs=3)
                            for tp in range(NPAIR):
                                nc.tensor.matmul(
                                    oo,
                                    lhsT=aot[
                                        :, tp * S + i * P : tp * S + (i + 1) * P
                                    ],
                                    rhs=wo_sb[
                                        :, tp * D + c * 512 : tp * D + (c + 1) * 512
                                    ],
                                    start=(tp == 0),
                                    stop=(tp == NPAIR - 1),
                                )
                            nc.vector.tensor_copy(
                                osb[:, c * 512 : (c + 1) * 512], oo
                            )
                        nc.sync.dma_start(
                            out=out_d[i * P : (i + 1) * P, :], in_=osb
                        )
            pull(10 ** 6)  # drain any remaining proj work for pair t+1

        if dbg is not None:
            nc.gpsimd.dma_start(out=dbg["qt"][:], in_=qt)
            nc.gpsimd.dma_start(out=dbg["kt"][:], in_=kt)
            nc.gpsimd.dma_start(out=dbg["v4"][:], in_=v4)
            nc.gpsimd.dma_start(out=dbg["aot"][:], in_=aot)


def work_tile(pool, tag, shape, dtype, bufs):
    wt = pool.tile(shape, dtype, tag=tag, bufs=bufs, name=tag)
    return wt


def build_nc(debug=False):
    nc = bass.Bass()
    x = nc.dram_tensor("x", [S, D], F32, kind="ExternalInput")
    wq_d = nc.dram_tensor("wq", [NH * H, D], F32, kind="ExternalInput")
    wk_d = nc.dram_tensor("wk", [NH * H, D], F32, kind="ExternalInput")
    wv_d = nc.dram_tensor("wv", [NH * H, D], F32, kind="ExternalInput")
    wo_d = nc.dram_tensor("wo", [NH * H, D], F32, kind="ExternalInput")
    out_d = nc.dram_tensor("out", [S, D], F32, kind="ExternalOutput")
    dbg = None
    if debug:
        dbg = {
            "xt": nc.dram_tensor("dbg_xt", [P, DT * S], F32, kind="ExternalOutput"),
            "wqt": nc.dram_tensor("dbg_wqt", [P, DT * 512], F32, kind="ExternalOutput"),
            "qt": nc.dram_tensor("dbg_qt", [P, NPAIR * S], F32, kind="ExternalOutput"),
            "kt": nc.dram_tensor("dbg_kt", [P, NPAIR * S], F32, kind="ExternalOutput"),
            "v4": nc.dram_tensor("dbg_v4", [P, ST * NH * 65], F32, kind="ExternalOutput"),
            "aot": nc.dram_tensor("dbg_aot", [P, NPAIR * S], F32, kind="ExternalOutput"),
        }
    with tile.TileContext(nc) as tc:
        emit(nc, tc, x, wq_d, wk_d, wv_d, wo_d, out_d, dbg=dbg)
    split_excess_waits(nc, compute_max_waits=COMPUTE_MAX_WAITS)
    return nc


_cache = {}


def _get_runner():
    """Persistent jitted 8-core runner (mirrors bass2jax.run_bass_via_pjrt's
    multi-core path, but reusable across calls so we can time executions)."""
    if "runner" in _cache:
        return _cache["runner"]
    import jax
    from jax.experimental.shard_map import shard_map
    from jax.sharding import Mesh, PartitionSpec
    from concourse import bass2jax

    bass2jax.install_neuronx_cc_hook()
    if "nc" not in _cache:
        _cache["nc"] = build_nc()
    nc = _cache["nc"]

    partition_name = nc.partition_id_tensor.name if nc.partition_id_tensor else None
    in_names, out_names, out_avals = [], [], []
    for alloc in nc.m.functions[0].allocations:
        if not isinstance(alloc, mybir.MemoryLocationSet):
            continue
        name = alloc.memorylocations[0].name
        if alloc.kind == "ExternalInput":
            if name != partition_name:
                in_names.append(name)
        elif alloc.kind == "ExternalOutput":
            out_names.append(name)
            out_avals.append(
                jax.core.ShapedArray(tuple(alloc.tensor_shape), mybir.dt.np(alloc.dtype))
            )
    n_params, n_outs = len(in_names), len(out_names)
    all_names = list(in_names) + list(out_names)
    if partition_name is not None:
        all_names.append(partition_name)
    all_names = tuple(all_names)

    def _body(*args):
        operands = list(args)
        if partition_name is not None:
            operands.append(bass2jax.partition_id_tensor())
        outs = bass2jax._bass_exec_p.bind(
            *operands,
            out_avals=tuple(out_avals),
            in_names=all_names,
            out_names=tuple(out_names),
            lowering_input_output_aliases=(),
            sim_require_finite=True,
            sim_require_nnan=True,
            nc=nc,
        )
        return tuple(outs)

    devices = jax.devices()[:8]
    mesh = Mesh(np.asarray(devices), ("core",))
    in_specs = (PartitionSpec("core"),) * (n_params + n_outs)
    out_specs = (PartitionSpec("core"),) * n_outs
    donate = tuple(range(n_params, n_params + n_outs))
    sharded = jax.jit(
        shard_map(_body, mesh=mesh, in_specs=in_specs, out_specs=out_specs, check_rep=False),
        donate_argnums=donate,
        keep_unused=True,
    )
    _cache["runner"] = (sharded, in_names, out_names, out_avals, mesh)
    return _cache["runner"]


def run_on_cores(in_maps):
    """Run the kernel on 8 cores; returns list of per-core output dicts."""
    sharded, in_names, out_names, out_avals, mesh = _get_runner()
    concat_in = [
        np.concatenate([np.asarray(in_maps[c][name]) for c in range(8)], axis=0)
        for name in in_names
    ]
    concat_zeros = [
        np.zeros((8 * a.shape[0], *a.shape[1:]), a.dtype) for a in out_avals
    ]
    out_arrs = sharded(*concat_in, *concat_zeros)
    return [
        {
            name: np.asarray(out_arrs[i]).reshape(8, *out_avals[i].shape)[c]
            for i, name in enumerate(out_names)
        }
        for c in range(8)
    ]


def make_in_maps(residual, W_Q, W_K, W_V, W_O):
    in_maps = []
    for core in range(8):
        b, g = core // 2, core % 2
        sl = slice(8 * g, 8 * (g + 1))
        in_maps.append(
            {
                "x": np.ascontiguousarray(residual[b], dtype=np.float32),
                "wq": np.ascontiguousarray(W_Q[sl].reshape(NH * H, D), dtype=np.float32),
                "wk": np.ascontiguousarray(W_K[sl].reshape(NH * H, D), dtype=np.float32),
                "wv": np.ascontiguousarray(W_V[sl].reshape(NH * H, D), dtype=np.float32),
                "wo": np.ascontiguousarray(W_O[sl].reshape(NH * H, D), dtype=np.float32),
            }
        )
    return in_maps


def kernel(residual, W_Q, W_K, W_V, W_O):
    residual = np.asarray(residual)
    in_maps = make_in_maps(residual, W_Q, W_K, W_V, W_O)
    results = run_on_cores(in_maps)
    B = residual.shape[0]
    out = np.zeros((B, S, D), np.float32)
    for core in range(8):
        b = core // 2
        out[b] += results[core]["out"]
    return out


if __name__ == "__main__":
    rng = np.random.default_rng(0)
    residual = rng.standard_normal((4, S, D)).astype(np.float32)
    W = [0.02 * rng.standard_normal((16, H, D)).astype(np.float32) for _ in range(4)]
    out = kernel(residual, *W)
    print("kernel ran, out shape", out.shape, "finite:", np.isfinite(out).all())
